# revision 9
# baseline (speedup 1.0000x reference)
"""Trainium2 Bass kernel: MultiHeadCrossAttentionWithBias.

Reference computation (per batch b):
  q_u = scale*(u_enc @ wq + wq_b); k/v from e_enc (and vice versa)
  ue_w = softmax(q_u k_e^T + bpp + mask*-inf); u_ctx = ue_w @ v_e
  u_update = u_ctx @ wo + wo_b                     (same mirrored for e)

Sharding: the problem decomposes into 8 fully independent attention units:
(batch b, direction d) for b in 0..3, d in {u->e, e->u}. Core i = (d, b)
handles one unit end-to-end; no collectives needed.

Per-core layouts (host prepares pure layout transforms only — transposes /
slices; all arithmetic runs on device):
  encQT  [D=512, L=1024]  query-side encoder, transposed
  encKT  [D=512, L=1024]  key-side encoder, transposed
  bpp    [L, L]           logit bias orientated [k, q]
  mask   [L, L] uint8     mask orientated [k, q]

On-device math (per core):
  qT[f, s] = scale*(wq^T encQT + wq_b)   (f = h*64+hd on partitions)
  kT[f, s] =        wk^T encKT + wk_b
  v[s, f]  =        encKT^T wv + wv_b    (+ fused ones column per head)
  CB[k, q] = bpp_w*bpp + bpp_b + (mask-1)*1e30
  per head h: S^T[k,q] = CB + kT_h^T qT_h   (CB preloaded into PSUM via
              identity matmul, QK accumulates on top)
              E = exp(S^T)               (no max-subtraction: logits are
                                          O(10) for this distribution)
              [ctx^T; den] = [v_h | 1]^T E   (PSUM accumulation over k)
              ctxn_h = ctx^T * (1/den)   (denominator broadcast via DRAM
                                          bounce with partition-step-0 load)
  out[s, e] = wo_b + sum_h ctxn_h^T wo_h
"""

import numpy as np
from contextlib import ExitStack

import concourse.bass as bass
import concourse.tile as tile
import concourse.bacc as bacc
import concourse.mybir as mybir
from concourse.masks import make_identity
from concourse import bass_utils

F32 = mybir.dt.float32
F32R = mybir.dt.float32r
U8 = mybir.dt.uint8
AF = mybir.ActivationFunctionType
ALU = mybir.AluOpType

B, L, D, H, HD = 4, 1024, 512, 8, 64
P = 128
FH = H * HD            # 512
SCALE = 1.0 / np.sqrt(HD)
NEG = -1.0e30
N_CORES = 8


def bcast_ap(dram_ap, parts):
    """Partition-step-0 broadcast AP over a DRAM row."""
    return bass.AP(tensor=dram_ap.tensor, offset=dram_ap.offset,
                   ap=[[0, parts]] + list(dram_ap.ap))


def build_module():
    nc = bacc.Bacc("TRN2", target_bir_lowering=False, debug=False)

    encQT_d = nc.dram_tensor("encQT", [D, L], F32, kind="ExternalInput")
    encKT_d = nc.dram_tensor("encKT", [D, L], F32, kind="ExternalInput")
    bpp_d = nc.dram_tensor("bpp", [L, L], F32, kind="ExternalInput")
    mask_d = nc.dram_tensor("mask", [L, L], U8, kind="ExternalInput")
    wq_d = nc.dram_tensor("wq", [D, FH], F32, kind="ExternalInput")
    wk_d = nc.dram_tensor("wk", [D, FH], F32, kind="ExternalInput")
    wv_d = nc.dram_tensor("wv", [D, FH], F32, kind="ExternalInput")
    wo_d = nc.dram_tensor("wo", [FH, D], F32, kind="ExternalInput")
    wqb_d = nc.dram_tensor("wqb", [FH], F32, kind="ExternalInput")
    wkb_d = nc.dram_tensor("wkb", [FH], F32, kind="ExternalInput")
    wvb_d = nc.dram_tensor("wvb", [FH], F32, kind="ExternalInput")
    wob_d = nc.dram_tensor("wob", [D], F32, kind="ExternalInput")
    bppw_d = nc.dram_tensor("bppw", [1, 1], F32, kind="ExternalInput")
    bppb_d = nc.dram_tensor("bppb", [1, 1], F32, kind="ExternalInput")
    out_d = nc.dram_tensor("out", [L, D], F32, kind="ExternalOutput")
    den_d = nc.dram_tensor("den_scratch", [H, L], F32, kind="Internal")

    with tile.TileContext(nc) as tc, ExitStack() as ctx:
        const = ctx.enter_context(tc.tile_pool(name="const", bufs=1))
        qkT_p = ctx.enter_context(tc.tile_pool(name="qkT", bufs=8))
        v_p = ctx.enter_context(tc.tile_pool(name="v", bufs=8))
        wo_p = ctx.enter_context(tc.tile_pool(name="wo", bufs=8))
        ps_s = ctx.enter_context(tc.tile_pool(name="ps_s", bufs=2, space="PSUM"))
        ps_c = ctx.enter_context(tc.tile_pool(name="ps_c", bufs=4, space="PSUM"))

        # ---- constants / small prep ----
        ident_f = const.tile([P, P], F32)
        make_identity(nc, ident_f[:])
        ident = const.tile([P, P], F32R)
        nc.vector.tensor_copy(ident[:], ident_f[:])
        ones_col = const.tile([1, P], F32R)
        nc.vector.memset(ones_col[:].bitcast(F32), 1.0)
        wob_f = const.tile([1, D], F32)
        nc.sync.dma_start(wob_f[:], wob_d.ap().rearrange("(o e) -> o e", o=1))
        wob_sb = const.tile([1, D], F32R)
        nc.vector.tensor_copy(wob_sb[:], wob_f[:])
        # bpp_w / bpp_b broadcast to [128,1] columns
        bw_col = const.tile([P, 1], F32)
        nc.gpsimd.dma_start(bw_col[:], bcast_ap(bppw_d.ap()[0:1, :], P))
        bb_col = const.tile([P, 1], F32)
        nc.gpsimd.dma_start(bb_col[:], bcast_ap(bppb_d.ap()[0:1, :], P))
        # mask-conversion bias: bpp_b - 1e30 per partition
        bmb = const.tile([P, 1], F32)
        nc.vector.tensor_scalar_add(bmb[:], bb_col[:], NEG)
        # projection biases
        wqb_raw = const.tile([P, 4], F32)
        nc.sync.dma_start(wqb_raw[:], wqb_d.ap().rearrange("(c p) -> p c", p=P))
        wqb_sc = const.tile([P, 4], F32)
        nc.vector.tensor_scalar_mul(wqb_sc[:], wqb_raw[:], float(SCALE))
        wkb_c = const.tile([P, 4], F32)
        nc.sync.dma_start(wkb_c[:], wkb_d.ap().rearrange("(c p) -> p c", p=P))
        wvb_bc = const.tile([P, FH], F32)
        nc.gpsimd.dma_start(wvb_bc[:], bcast_ap(wvb_d.ap(), P))
        # wo per head [64, 512] (staged f32 -> rounded f32r)
        wo_t = []
        with tc.tile_pool(name="wostage", bufs=3) as wos_p:
            for h in range(H):
                s = wos_p.tile([HD, D], F32, tag="wos", name=f"wos{h}")
                nc.sync.dma_start(s[:], wo_d.ap()[h * HD:(h + 1) * HD, :])
                t = wo_p.tile([HD, D], F32R, tag="wo", name=f"wo{h}")
                nc.gpsimd.tensor_copy(t[:], s[:])
                wo_t.append(t)

        # ---- projections (enc + proj weights live only here) ----
        qT, kT, v_aug = [], [], []
        with tc.tile_pool(name="enc", bufs=8) as enc_p, \
             tc.tile_pool(name="wqkv", bufs=12) as w_p:
            eq, ek = [], []
            wq_t, wk_t, wv_t = [], [], []
            with tc.tile_pool(name="stage", bufs=2) as st_p:
                for lst, dram in ((eq, encQT_d), (ek, encKT_d)):
                    for dc in range(4):
                        s = st_p.tile([P, L], F32, tag="st",
                                      name=f"st_{dram.name}{dc}")
                        nc.sync.dma_start(s[:], dram.ap()[dc * P:(dc + 1) * P, :])
                        t = enc_p.tile([P, L], F32R, tag="enc",
                                       name=f"enc_{dram.name}{dc}")
                        nc.gpsimd.tensor_copy(t[:], s[:])
                        lst.append(t)
                for w_dram, lst in ((wq_d, wq_t), (wk_d, wk_t), (wv_d, wv_t)):
                    for dc in range(4):
                        s = st_p.tile([P, FH], F32, tag="stw",
                                      name=f"stw_{w_dram.name}{dc}")
                        nc.sync.dma_start(s[:], w_dram.ap()[dc * P:(dc + 1) * P, :])
                        t = w_p.tile([P, FH], F32R, tag="w",
                                     name=f"w_{w_dram.name}{dc}")
                        nc.gpsimd.tensor_copy(t[:], s[:])
                        lst.append(t)

            # qT / kT: [f, s] packed two heads per 128-partition chunk
            for which, w_t, enc_t, out_list in (
                ("q", wq_t, eq, qT), ("k", wk_t, ek, kT),
            ):
                for pc in range(4):
                    ps = ps_s.tile([P, L], F32, tag="ps_s")
                    for sh in range(2):
                        for dc in range(4):
                            nc.tensor.matmul(
                                ps[:, sh * 512:(sh + 1) * 512],
                                w_t[dc][:, pc * P:(pc + 1) * P],
                                enc_t[dc][:, sh * 512:(sh + 1) * 512],
                                start=(dc == 0), stop=(dc == 3))
                    o = qkT_p.tile([P, L], F32R, tag="qkT")
                    if which == "q":
                        nc.scalar.activation(o[:], ps[:], AF.Identity,
                                             bias=wqb_sc[:, pc:pc + 1],
                                             scale=float(SCALE))
                    else:
                        nc.scalar.activation(o[:], ps[:], AF.Identity,
                                             bias=wkb_c[:, pc:pc + 1], scale=1.0)
                    out_list.append(o)

            # v: [s, f] with ones column interleaved per head ([128, 8*65])
            for sc in range(8):
                ps = ps_s.tile([P, 512], F32, tag="ps_s")
                for dc in range(4):
                    nc.tensor.matmul(ps[:], ek[dc][:, sc * P:(sc + 1) * P],
                                     wv_t[dc][:], start=(dc == 0),
                                     stop=(dc == 3))
                va = v_p.tile([P, H * (HD + 1)], F32R, tag="v")
                vg = va[:].rearrange("p (h c) -> p h c", c=HD + 1)
                nc.vector.scalar_tensor_tensor(
                    vg[:, :, 0:HD],
                    ps[:].rearrange("p (h c) -> p h c", c=HD), 1.0,
                    wvb_bc[:].rearrange("p (h c) -> p h c", c=HD),
                    ALU.bypass, ALU.add)
                nc.vector.memset(vg[:, :, HD:HD + 1].bitcast(F32), 1.0)
                v_aug.append(va)

        # ---- combined bias CB[k, q] = bpp_w*bpp + bpp_b + (mask-1)*1e30 ----
        cb_p = ctx.enter_context(tc.tile_pool(name="cb", bufs=8))
        cb = []
        with tc.tile_pool(name="cbtmp", bufs=4) as cbt_p:
            for kc in range(8):
                m_t = cbt_p.tile([P, L], U8, tag="m")
                nc.sync.dma_start(m_t[:], mask_d.ap()[kc * P:(kc + 1) * P, :])
                mf = cbt_p.tile([P, L], F32, tag="mf")
                nc.scalar.activation(mf[:], m_t[:], AF.Identity,
                                     bias=bmb[:], scale=-NEG)
                b_t = cbt_p.tile([P, L], F32, tag="b")
                nc.sync.dma_start(b_t[:], bpp_d.ap()[kc * P:(kc + 1) * P, :])
                c_t = cb_p.tile([P, L], F32R, tag="cb")
                nc.vector.scalar_tensor_tensor(
                    c_t[:], b_t[:], bw_col[:, 0:1], mf[:], ALU.mult, ALU.add)
                cb.append(c_t)

        # ---- attention ----
        ctxT_p = ctx.enter_context(tc.tile_pool(name="ctxT", bufs=8))
        ctxT = []
        with tc.tile_pool(name="e", bufs=3) as e_p, \
             tc.tile_pool(name="rec", bufs=2) as rec_p, \
             tc.tile_pool(name="rb", bufs=2) as rb_p:
            for h in range(H):
                o = (h % 2) * HD
                pc = h // 2
                c_ps = [ps_c.tile([HD + 1, 512], F32, tag="ps_c",
                                  name=f"c_ps_{h}_{i}")
                        for i in range(2)]
                for kc in range(8):
                    s_ps = ps_s.tile([P, L], F32, tag="ps_s")
                    for qh in range(2):
                        sl = slice(qh * 512, (qh + 1) * 512)
                        nc.tensor.matmul(s_ps[:, sl], ident[:],
                                         cb[kc][:, sl],
                                         start=True, stop=False)
                        nc.tensor.matmul(
                            s_ps[:, sl],
                            kT[pc][o:o + HD, kc * P:(kc + 1) * P],
                            qT[pc][o:o + HD, sl],
                            start=False, stop=True)
                    e_t = e_p.tile([P, L], F32R, tag="e")
                    nc.scalar.activation(e_t[:], s_ps[:], AF.Exp)
                    for qh in range(2):
                        sl = slice(qh * 512, (qh + 1) * 512)
                        nc.tensor.matmul(
                            c_ps[qh][:],
                            v_aug[kc][:, h * (HD + 1):(h + 1) * (HD + 1)],
                            e_t[:, sl],
                            start=(kc == 0), stop=(kc == 7))
                rec = rec_p.tile([HD + 1, L], F32, tag="rec")
                for qh in range(2):
                    nc.vector.reciprocal(
                        rec[HD:HD + 1, qh * 512:(qh + 1) * 512],
                        c_ps[qh][HD:HD + 1, :])
                nc.sync.dma_start(den_d.ap()[h:h + 1, :], rec[HD:HD + 1, :])
                rb = rb_p.tile([HD, L], F32, tag="rb")
                nc.gpsimd.dma_start(rb[:], bcast_ap(den_d.ap()[h:h + 1, :], HD))
                ct = ctxT_p.tile([HD, L], F32R, tag="ctxT")
                for qh in range(2):
                    sl = slice(qh * 512, (qh + 1) * 512)
                    nc.vector.scalar_tensor_tensor(
                        ct[:, sl], c_ps[qh][0:HD, :], 1.0, rb[:, sl],
                        ALU.bypass, ALU.mult)
                ctxT.append(ct)

        # ---- output projection ----
        with tc.tile_pool(name="outp", bufs=3) as out_p:
            for st in range(8):
                o_ps = ps_s.tile([P, D], F32, tag="ps_s")
                nc.tensor.matmul(o_ps[:], ones_col[:], wob_sb[:],
                                 start=True, stop=False)
                for h in range(H):
                    nc.tensor.matmul(o_ps[:],
                                     ctxT[h][:, st * P:(st + 1) * P],
                                     wo_t[h][:],
                                     start=False, stop=(h == 7))
                o_t = out_p.tile([P, D], F32, tag="out")
                nc.vector.tensor_copy(o_t[:], o_ps[:])
                nc.sync.dma_start(out_d.ap()[st * P:(st + 1) * P, :], o_t[:])

    nc.compile()
    return nc


def shard_inputs(u_enc, e_enc, logit_bpp, ue_mask, eu_mask,
                 wq_k, wq_b, wk_k, wk_b, wv_k, wv_b, wo_k, wo_b,
                 bpp_w, bpp_b):
    """Build the 8 per-core input maps (pure layout transforms)."""
    u_enc = np.asarray(u_enc, np.float32)
    e_enc = np.asarray(e_enc, np.float32)
    bpp = np.asarray(logit_bpp, np.float32)
    ue_m = np.asarray(ue_mask).astype(np.uint8)
    eu_m = np.asarray(eu_mask).astype(np.uint8)
    com = dict(
        wq=np.ascontiguousarray(np.asarray(wq_k, np.float32).reshape(D, FH)),
        wk=np.ascontiguousarray(np.asarray(wk_k, np.float32).reshape(D, FH)),
        wv=np.ascontiguousarray(np.asarray(wv_k, np.float32).reshape(D, FH)),
        wo=np.ascontiguousarray(np.asarray(wo_k, np.float32).reshape(FH, D)),
        wqb=np.asarray(wq_b, np.float32).reshape(FH).copy(),
        wkb=np.asarray(wk_b, np.float32).reshape(FH).copy(),
        wvb=np.asarray(wv_b, np.float32).reshape(FH).copy(),
        wob=np.asarray(wo_b, np.float32).reshape(D).copy(),
        bppw=np.asarray(bpp_w, np.float32).reshape(1, 1).copy(),
        bppb=np.asarray(bpp_b, np.float32).reshape(1, 1).copy(),
    )
    bppT = np.ascontiguousarray(bpp.T)
    in_maps = []
    for i in range(N_CORES):
        d, b = divmod(i, B)
        if d == 0:      # u queries, e keys -> u_update[b]
            m = dict(encQT=np.ascontiguousarray(u_enc[b].T),
                     encKT=np.ascontiguousarray(e_enc[b].T),
                     bpp=bppT,
                     mask=np.ascontiguousarray(ue_m[b, 0].T))
        else:           # e queries, u keys -> e_update[b]
            m = dict(encQT=np.ascontiguousarray(e_enc[b].T),
                     encKT=np.ascontiguousarray(u_enc[b].T),
                     bpp=bpp,
                     mask=np.ascontiguousarray(eu_m[b, 0].T))
        m.update(com)
        in_maps.append(m)
    return in_maps


_NC = None


def kernel(**inputs):
    global _NC
    if _NC is None:
        _NC = build_module()
    in_maps = shard_inputs(**inputs)
    res = bass_utils.run_bass_kernel_spmd(
        _NC, in_maps, core_ids=list(range(N_CORES)))
    u_update = np.stack([res.results[b]["out"] for b in range(B)])
    e_update = np.stack([res.results[B + b]["out"] for b in range(B)])
    return u_update, e_update


if __name__ == "__main__":
    # single-core CoreSim check of one (direction, batch) unit
    from concourse.bass_interp import CoreSim

    rng = np.random.default_rng(0)
    u = rng.standard_normal((B, L, D), np.float32)
    e = rng.standard_normal((B, L, D), np.float32)
    bpp = rng.standard_normal((L, L), np.float32)
    uem = (rng.random((B, 1, L, L)) < 0.9)
    eum = (rng.random((B, 1, L, L)) < 0.9)
    w = 1.0 / np.sqrt(D)
    wq = (rng.standard_normal((D, H, HD), np.float32) * w)
    wk = (rng.standard_normal((D, H, HD), np.float32) * w)
    wv = (rng.standard_normal((D, H, HD), np.float32) * w)
    wo = (rng.standard_normal((H, HD, D), np.float32) / np.sqrt(FH))
    zq = rng.standard_normal((H, HD), np.float32) * 0.1
    zo = rng.standard_normal((D,), np.float32) * 0.1

    nc = build_module()
    in_maps = shard_inputs(u, e, bpp, uem, eum, wq, zq, wk, zq, wv, zq,
                           wo, zo, np.float32(1.3), np.float32(-0.2))

    core = 0
    sim = CoreSim(nc, trace=False)
    for k, vv in in_maps[core].items():
        sim.tensor(k)[:] = vv
    sim.simulate(check_with_hw=False)
    got = np.array(sim.tensor("out"))

    # numpy reference for that unit
    def ref_unit(encQ, encK, bias_qk, mask_qk):
        q = SCALE * (encQ @ wq.reshape(D, FH) + zq.reshape(FH))
        kk = encK @ wk.reshape(D, FH) + zq.reshape(FH)
        vv = encK @ wv.reshape(D, FH) + zq.reshape(FH)
        outs = np.zeros((L, D), np.float32)
        accum = np.zeros((L, D), np.float64)
        for h in range(H):
            qi = q[:, h * HD:(h + 1) * HD]
            ki = kk[:, h * HD:(h + 1) * HD]
            vi = vv[:, h * HD:(h + 1) * HD]
            s = qi @ ki.T + bias_qk
            s = np.where(mask_qk, s, -np.inf)
            s = s - s.max(-1, keepdims=True)
            p_ = np.exp(s)
            p_ /= p_.sum(-1, keepdims=True)
            accum += (p_ @ vi) @ wo[h]
        outs = (accum + zo).astype(np.float32)
        return outs

    bq = 1.3 * bpp + -0.2
    exp_out = ref_unit(u[0], e[0], bq, uem[0, 0])
    err = np.abs(got - exp_out).max() / np.abs(exp_out).max()
    print("unit relerr vs numpy:", err)


# revision 14
# speedup vs baseline: 1.0446x; 1.0446x over previous
"""Trainium2 Bass kernel: MultiHeadCrossAttentionWithBias.

Reference computation (per batch b):
  q_u = scale*(u_enc @ wq + wq_b); k/v from e_enc (and vice versa)
  ue_w = softmax(q_u k_e^T + bpp + mask*-inf); u_ctx = ue_w @ v_e
  u_update = u_ctx @ wo + wo_b                     (same mirrored for e)

Sharding: the problem decomposes into 8 fully independent attention units:
(batch b, direction d) for b in 0..3, d in {u->e, e->u}. Core i = (d, b)
handles one unit end-to-end; no collectives needed.

Host prep is layout/precision only (transposes, slices, fp32->fp32r
rounding of matmul operands); all FLOPs run on device.

Per-core inputs:
  encQT  [D=512, L=1024] f32r  query-side encoder, transposed
  encKT  [D=512, L=1024] f32r  key-side encoder, transposed
  bpp    [L, L] f32            logit bias oriented [k, q]
  mask   [L, L] uint8          mask oriented [k, q]
  wq/wk/wv [D, 512] f32r, wo [512, D] f32r, biases f32

On-device math (per core):
  qT[f, s] = scale*(wq^T encQT + wq_b)   (f = h*64+hd on partitions)
  kT[f, s] =        wk^T encKT + wk_b
  v[s, f]  =        encKT^T wv + wv_b    (+ fused ones column per head)
  CB[k, q] = bpp_w*bpp + bpp_b + (mask-1)*1e30      (gpsimd)
  per head h, k-chunk kc:
      S^T[k,q] = kT_h^T qT_h       (PE, PSUM)
      S^T += CB[kc]                (DVE scalar_tensor_tensor in-place)
      E = exp(S^T)                 (ACT; no max-subtraction: logits O(10))
      [ctx^T; den] += [v_h | 1]^T E  (PE, PSUM accumulation over kc)
  rcp = approx_reciprocal(den); broadcast via DRAM bounce
  ctxn[pair] = ctx^T * rcp  (DVE, odd head written to partitions 64..127)
  out[s, e] = sum_pair ctxn_p^T wo_p + wo_b   (PE + DVE bias-add eviction)
"""

import numpy as np
from contextlib import ExitStack

import concourse.bass as bass
import concourse.tile as tile
import concourse.bacc as bacc
import concourse.mybir as mybir
from concourse import bass_utils

F32 = mybir.dt.float32
F32R = mybir.dt.float32r
U8 = mybir.dt.uint8
AF = mybir.ActivationFunctionType
ALU = mybir.AluOpType

B, L, D, H, HD = 4, 1024, 512, 8, 64
P = 128
FH = H * HD            # 512
SCALE = 1.0 / np.sqrt(HD)
NEG = -1.0e30
N_CORES = 8

# debug switches (HW bisect)
INPLACE_STT = True      # S^T += CB in place on PSUM (False: via SBUF tile)


def bcast_ap(dram_ap, parts):
    """Partition-step-0 broadcast AP over a DRAM row."""
    return bass.AP(tensor=dram_ap.tensor, offset=dram_ap.offset,
                   ap=[[0, parts]] + list(dram_ap.ap))


def round_f32r(x):
    """Round fp32 -> fp32r (11-bit mantissa, RNE), keeping fp32 layout."""
    u = np.ascontiguousarray(x, np.float32).view(np.uint32)
    r = (u + 0x7FF + ((u >> 12) & 1)) & np.uint32(0xFFFFF000)
    return r.view(np.float32)


def build_module():
    nc = bacc.Bacc("TRN2", target_bir_lowering=False, debug=False)

    encQT_d = nc.dram_tensor("encQT", [D, L], F32R, kind="ExternalInput")
    encKT_d = nc.dram_tensor("encKT", [D, L], F32R, kind="ExternalInput")
    wq_d = nc.dram_tensor("wq", [D, FH], F32R, kind="ExternalInput")
    wk_d = nc.dram_tensor("wk", [D, FH], F32R, kind="ExternalInput")
    wv_d = nc.dram_tensor("wv", [D, FH], F32R, kind="ExternalInput")
    wo_d = nc.dram_tensor("wo", [FH, D], F32R, kind="ExternalInput")
    bpp_d = nc.dram_tensor("bpp", [L, L], F32, kind="ExternalInput")
    mask_d = nc.dram_tensor("mask", [L, L], U8, kind="ExternalInput")
    wqb_d = nc.dram_tensor("wqb", [FH], F32, kind="ExternalInput")
    wkb_d = nc.dram_tensor("wkb", [FH], F32, kind="ExternalInput")
    wvb_d = nc.dram_tensor("wvb", [FH], F32, kind="ExternalInput")
    wob_d = nc.dram_tensor("wob", [D], F32, kind="ExternalInput")
    bppw_d = nc.dram_tensor("bppw", [1, 1], F32, kind="ExternalInput")
    bppb_d = nc.dram_tensor("bppb", [1, 1], F32, kind="ExternalInput")
    out_d = nc.dram_tensor("out", [L, D], F32, kind="ExternalOutput")
    den_d = nc.dram_tensor("den_scratch", [H, L], F32, kind="Internal")

    with tile.TileContext(nc) as tc, ExitStack() as ctx:
        const = ctx.enter_context(tc.tile_pool(name="const", bufs=1))
        qkT_p = ctx.enter_context(tc.tile_pool(name="qkT", bufs=8))
        v_p = ctx.enter_context(tc.tile_pool(name="v", bufs=8))
        wo_p = ctx.enter_context(tc.tile_pool(name="wo", bufs=4))
        cb_p = ctx.enter_context(tc.tile_pool(name="cb", bufs=8))
        ps_s = ctx.enter_context(tc.tile_pool(name="ps_s", bufs=2, space="PSUM"))
        ps_c = ctx.enter_context(tc.tile_pool(name="ps_c", bufs=4, space="PSUM"))

        # ---- weight/bias prep ----
        wo_t = []
        for p_ in range(4):
            t = wo_p.tile([P, D], F32R, tag="wo", name=f"wo{p_}")
            nc.sync.dma_start(t[:], wo_d.ap()[p_ * P:(p_ + 1) * P, :])
            wo_t.append(t)
        # bpp_w / bpp_b broadcast to [128,1] columns
        bw_col = const.tile([P, 1], F32)
        nc.gpsimd.dma_start(bw_col[:], bcast_ap(bppw_d.ap()[0:1, :], P))
        bb_col = const.tile([P, 1], F32)
        nc.gpsimd.dma_start(bb_col[:], bcast_ap(bppb_d.ap()[0:1, :], P))
        bmb = const.tile([P, 1], F32)
        nc.vector.tensor_scalar_add(bmb[:], bb_col[:], NEG)
        # projection biases
        wqb_raw = const.tile([P, 4], F32)
        nc.sync.dma_start(wqb_raw[:], wqb_d.ap().rearrange("(c p) -> p c", p=P))
        wqb_sc = const.tile([P, 4], F32)
        nc.vector.tensor_scalar_mul(wqb_sc[:], wqb_raw[:], float(SCALE))
        wkb_c = const.tile([P, 4], F32)
        nc.sync.dma_start(wkb_c[:], wkb_d.ap().rearrange("(c p) -> p c", p=P))
        wvb_bc = const.tile([P, FH], F32)
        nc.gpsimd.dma_start(wvb_bc[:], bcast_ap(wvb_d.ap(), P))
        wob_bc = const.tile([P, D], F32)
        nc.gpsimd.dma_start(wob_bc[:], bcast_ap(wob_d.ap(), P))

        # ---- combined bias CB[k, q] (gpsimd; overlaps projections) ----
        cb = []
        cbt_p = tc.alloc_tile_pool(name="cbtmp", bufs=3)
        for kc in range(8):
            m_t = cbt_p.tile([P, L], U8, tag="m", name=f"m{kc}")
            nc.sync.dma_start(m_t[:], mask_d.ap()[kc * P:(kc + 1) * P, :])
            mf = cbt_p.tile([P, L], F32, tag="mf", name=f"mf{kc}")
            nc.scalar.activation(mf[:], m_t[:], AF.Identity,
                                 bias=bmb[:], scale=-NEG)
            b_t = cbt_p.tile([P, L], F32, tag="b", name=f"b{kc}")
            nc.sync.dma_start(b_t[:], bpp_d.ap()[kc * P:(kc + 1) * P, :])
            c_t = cb_p.tile([P, L], F32, tag="cb", name=f"cb{kc}")
            nc.vector.scalar_tensor_tensor(
                c_t[:], b_t[:], bw_col[:, 0:1], mf[:], ALU.mult, ALU.add)
            cb.append(c_t)
        cbt_p.release()

        # ---- projections ----
        qT, kT, v_aug = [], [], []
        with tc.tile_pool(name="enc", bufs=8) as enc_p, \
             tc.tile_pool(name="wqkv", bufs=12) as w_p:
            eq, ek = [], []
            wq_t, wk_t, wv_t = [], [], []
            for w_dram, lst in ((wq_d, wq_t), (wk_d, wk_t), (wv_d, wv_t)):
                for dc in range(4):
                    t = w_p.tile([P, FH], F32R, tag="w",
                                 name=f"w_{w_dram.name}{dc}")
                    nc.sync.dma_start(t[:], w_dram.ap()[dc * P:(dc + 1) * P, :])
                    lst.append(t)
            for lst, dram in ((eq, encQT_d), (ek, encKT_d)):
                for dc in range(4):
                    t = enc_p.tile([P, L], F32R, tag="enc",
                                   name=f"enc_{dram.name}{dc}")
                    nc.sync.dma_start(t[:], dram.ap()[dc * P:(dc + 1) * P, :])
                    lst.append(t)

            # qT / kT: [f, s] packed two heads per 128-partition chunk
            for which, w_t, enc_t, out_list in (
                ("q", wq_t, eq, qT), ("k", wk_t, ek, kT),
            ):
                for pc in range(4):
                    ps = ps_s.tile([P, L], F32, tag="ps_s",
                                   name=f"ps_{which}{pc}")
                    for sh in range(2):
                        for dc in range(4):
                            nc.tensor.matmul(
                                ps[:, sh * 512:(sh + 1) * 512],
                                w_t[dc][:, pc * P:(pc + 1) * P],
                                enc_t[dc][:, sh * 512:(sh + 1) * 512],
                                start=(dc == 0), stop=(dc == 3))
                    o = qkT_p.tile([P, L], F32R, tag="qkT",
                                   name=f"{which}T{pc}")
                    if which == "q":
                        nc.scalar.activation(o[:], ps[:], AF.Identity,
                                             bias=wqb_sc[:, pc:pc + 1],
                                             scale=float(SCALE))
                    else:
                        nc.scalar.activation(o[:], ps[:], AF.Identity,
                                             bias=wkb_c[:, pc:pc + 1], scale=1.0)
                    out_list.append(o)

            # v: [s, f] with ones column interleaved per head ([128, 8*65])
            for sc in range(8):
                ps = ps_s.tile([P, 512], F32, tag="ps_s", name=f"ps_v{sc}")
                for dc in range(4):
                    nc.tensor.matmul(ps[:], ek[dc][:, sc * P:(sc + 1) * P],
                                     wv_t[dc][:], start=(dc == 0),
                                     stop=(dc == 3))
                va = v_p.tile([P, H * (HD + 1)], F32R, tag="v", name=f"v{sc}")
                vg = va[:].rearrange("p (h c) -> p h c", c=HD + 1)
                nc.vector.scalar_tensor_tensor(
                    vg[:, :, 0:HD],
                    ps[:].rearrange("p (h c) -> p h c", c=HD), 1.0,
                    wvb_bc[:].rearrange("p (h c) -> p h c", c=HD),
                    ALU.bypass, ALU.add)
                nc.vector.memset(vg[:, :, HD:HD + 1].bitcast(F32), 1.0)
                v_aug.append(va)

        # ---- attention ----
        ctxn_p = ctx.enter_context(tc.tile_pool(name="ctxn", bufs=4))
        ctxn = [None] * 4
        with tc.tile_pool(name="e", bufs=3) as e_p, \
             tc.tile_pool(name="rec", bufs=2) as rec_p, \
             tc.tile_pool(name="rb", bufs=2) as rb_p:
            for h in range(H):
                o = (h % 2) * HD
                pc = h // 2
                c_ps = [ps_c.tile([HD + 1, 512], F32, tag="ps_c",
                                  name=f"c_ps_{h}_{i}")
                        for i in range(2)]
                for kc in range(8):
                    s_ps = ps_s.tile([P, L], F32, tag="ps_s",
                                     name=f"s_ps_{h}_{kc}")
                    for qh in range(2):
                        sl = slice(qh * 512, (qh + 1) * 512)
                        nc.tensor.matmul(
                            s_ps[:, sl],
                            kT[pc][o:o + HD, kc * P:(kc + 1) * P],
                            qT[pc][o:o + HD, sl],
                            start=True, stop=True)
                    e_t = e_p.tile([P, L], F32R, tag="e", name=f"e_{h}_{kc}")
                    if INPLACE_STT:
                        nc.vector.scalar_tensor_tensor(
                            s_ps[:], s_ps[:], 1.0, cb[kc][:],
                            ALU.bypass, ALU.add)
                        nc.scalar.activation(e_t[:], s_ps[:], AF.Exp)
                    else:
                        sb_t = e_p.tile([P, L], F32, tag="sb",
                                        name=f"sb_{h}_{kc}")
                        nc.vector.scalar_tensor_tensor(
                            sb_t[:], s_ps[:], 1.0, cb[kc][:],
                            ALU.bypass, ALU.add)
                        nc.scalar.activation(e_t[:], sb_t[:], AF.Exp)
                    for qh in range(2):
                        sl = slice(qh * 512, (qh + 1) * 512)
                        nc.tensor.matmul(
                            c_ps[qh][:],
                            v_aug[kc][:, h * (HD + 1):(h + 1) * (HD + 1)],
                            e_t[:, sl],
                            start=(kc == 0), stop=(kc == 7))
                # denominators -> reciprocal (approx, ~2ulp) -> DRAM bounce
                rec = rec_p.tile([HD + 1, L], F32, tag="rec", name=f"rec{h}")
                scr = rec_p.tile([HD + 1, L], F32, tag="scr", name=f"scr{h}")
                for qh in range(2):
                    sl = slice(qh * 512, (qh + 1) * 512)
                    nc.vector.reciprocal(
                        rec[HD:HD + 1, sl], c_ps[qh][HD:HD + 1, :])
                nc.sync.dma_start(den_d.ap()[h:h + 1, :], rec[HD:HD + 1, :])
                rb = rb_p.tile([HD, L], F32, tag="rb", name=f"rb{h}")
                nc.gpsimd.dma_start(rb[:], bcast_ap(den_d.ap()[h:h + 1, :], HD))
                if h % 2 == 0:
                    ctxn[pc] = ctxn_p.tile([P, L], F32R, tag="ctxn",
                                           name=f"ctxn{pc}")
                for qh in range(2):
                    sl = slice(qh * 512, (qh + 1) * 512)
                    nc.vector.scalar_tensor_tensor(
                        ctxn[pc][o:o + HD, sl], c_ps[qh][0:HD, :], 1.0,
                        rb[:, sl], ALU.bypass, ALU.mult)

        # ---- output projection ----
        with tc.tile_pool(name="outp", bufs=3) as out_p:
            for st in range(8):
                o_ps = ps_s.tile([P, D], F32, tag="ps_s", name=f"o_ps{st}")
                for p_ in range(4):
                    nc.tensor.matmul(o_ps[:],
                                     ctxn[p_][:, st * P:(st + 1) * P],
                                     wo_t[p_][:],
                                     start=(p_ == 0), stop=(p_ == 3))
                o_t = out_p.tile([P, D], F32, tag="out", name=f"out{st}")
                nc.vector.scalar_tensor_tensor(
                    o_t[:], o_ps[:], 1.0, wob_bc[:], ALU.bypass, ALU.add)
                nc.sync.dma_start(out_d.ap()[st * P:(st + 1) * P, :], o_t[:])

    nc.compile()
    return nc


def shard_inputs(u_enc, e_enc, logit_bpp, ue_mask, eu_mask,
                 wq_k, wq_b, wk_k, wk_b, wv_k, wv_b, wo_k, wo_b,
                 bpp_w, bpp_b):
    """Build the 8 per-core input maps (layout + fp32r rounding only)."""
    u_enc = np.asarray(u_enc, np.float32)
    e_enc = np.asarray(e_enc, np.float32)
    bpp = np.asarray(logit_bpp, np.float32)
    ue_m = np.asarray(ue_mask).astype(np.uint8)
    eu_m = np.asarray(eu_mask).astype(np.uint8)
    com = dict(
        wq=round_f32r(np.asarray(wq_k, np.float32).reshape(D, FH)),
        wk=round_f32r(np.asarray(wk_k, np.float32).reshape(D, FH)),
        wv=round_f32r(np.asarray(wv_k, np.float32).reshape(D, FH)),
        wo=round_f32r(np.asarray(wo_k, np.float32).reshape(FH, D)),
        wqb=np.asarray(wq_b, np.float32).reshape(FH).copy(),
        wkb=np.asarray(wk_b, np.float32).reshape(FH).copy(),
        wvb=np.asarray(wv_b, np.float32).reshape(FH).copy(),
        wob=np.asarray(wo_b, np.float32).reshape(D).copy(),
        bppw=np.asarray(bpp_w, np.float32).reshape(1, 1).copy(),
        bppb=np.asarray(bpp_b, np.float32).reshape(1, 1).copy(),
    )
    uT = [round_f32r(u_enc[b].T) for b in range(B)]
    eT = [round_f32r(e_enc[b].T) for b in range(B)]
    bppT = np.ascontiguousarray(bpp.T)
    in_maps = []
    for i in range(N_CORES):
        d, b = divmod(i, B)
        if d == 0:      # u queries, e keys -> u_update[b]
            m = dict(encQT=uT[b], encKT=eT[b], bpp=bppT,
                     mask=np.ascontiguousarray(ue_m[b, 0].T))
        else:           # e queries, u keys -> e_update[b]
            m = dict(encQT=eT[b], encKT=uT[b], bpp=bpp,
                     mask=np.ascontiguousarray(eu_m[b, 0].T))
        m.update(com)
        in_maps.append(m)
    return in_maps


_NC = None


def kernel(**inputs):
    global _NC
    if _NC is None:
        _NC = build_module()
    in_maps = shard_inputs(**inputs)
    res = bass_utils.run_bass_kernel_spmd(
        _NC, in_maps, core_ids=list(range(N_CORES)))
    u_update = np.stack([res.results[b]["out"] for b in range(B)])
    e_update = np.stack([res.results[B + b]["out"] for b in range(B)])
    return u_update, e_update


if __name__ == "__main__":
    # single-core CoreSim check of one (direction, batch) unit
    from concourse.bass_interp import CoreSim

    rng = np.random.default_rng(0)
    u = rng.standard_normal((B, L, D)).astype(np.float32)
    e = rng.standard_normal((B, L, D)).astype(np.float32)
    bpp = rng.standard_normal((L, L)).astype(np.float32)
    uem = (rng.random((B, 1, L, L)) < 0.9)
    eum = (rng.random((B, 1, L, L)) < 0.9)
    w = 1.0 / np.sqrt(D)
    wq = (rng.standard_normal((D, H, HD)) * w).astype(np.float32)
    wk = (rng.standard_normal((D, H, HD)) * w).astype(np.float32)
    wv = (rng.standard_normal((D, H, HD)) * w).astype(np.float32)
    wo = (rng.standard_normal((H, HD, D)) / np.sqrt(FH)).astype(np.float32)
    zq = (rng.standard_normal((H, HD)) * 0.1).astype(np.float32)
    zo = (rng.standard_normal((D,)) * 0.1).astype(np.float32)

    nc = build_module()
    in_maps = shard_inputs(u, e, bpp, uem, eum, wq, zq, wk, zq, wv, zq,
                           wo, zo, np.float32(1.3), np.float32(-0.2))

    core = 0
    sim = CoreSim(nc, trace=False)
    for k, vv in in_maps[core].items():
        sim.tensor(k)[:] = vv
    sim.simulate(check_with_hw=False)
    got = np.array(sim.tensor("out"))

    def ref_unit(encQ, encK, bias_qk, mask_qk):
        q = SCALE * (encQ @ wq.reshape(D, FH) + zq.reshape(FH))
        kk = encK @ wk.reshape(D, FH) + zq.reshape(FH)
        vv = encK @ wv.reshape(D, FH) + zq.reshape(FH)
        accum = np.zeros((L, D), np.float64)
        for h in range(H):
            qi = q[:, h * HD:(h + 1) * HD]
            ki = kk[:, h * HD:(h + 1) * HD]
            vi = vv[:, h * HD:(h + 1) * HD]
            s = qi @ ki.T + bias_qk
            s = np.where(mask_qk, s, -np.inf)
            s = s - s.max(-1, keepdims=True)
            p_ = np.exp(s)
            p_ /= p_.sum(-1, keepdims=True)
            accum += (p_ @ vi) @ wo[h]
        return (accum + zo).astype(np.float32)

    bq = 1.3 * bpp + -0.2
    exp_out = ref_unit(u[0], e[0], bq, uem[0, 0])
    err = np.abs(got - exp_out).max() / np.abs(exp_out).max()
    print("unit relerr vs numpy:", err)
